# revision 1
# baseline (speedup 1.0000x reference)
"""CurricularFace loss kernel for 8 Trainium2 NeuronCores.

Strategy: tensor-parallel over out_features (classes). Each core owns a
12800-column shard of `kernel` (100000 padded to 102400) and computes its
[C_shard, N] slice of the S-scaled logits in transposed layout. The per-row
target path (target_logit, t update, cos_theta_m thresholds) is replicated on
every core from a host-gathered [D, N] matrix of target columns, so no
collectives are needed. The target-column scatter is applied on the host.

Math per core, in [c(partition), n(free)] layout:
  u      = (k_c . e_n) / ||k_c||                (unclipped cosine)
  r1     = relu(S - S*u)          (ACT, PSUM->SBUF, scale = -S/||k_c||)
  r2     = relu(2S - r1)          => cos_s = S*clip(u,-1,1) = r2 - S
  h      = (r2/sqrt(S) + sqrt(S)*(b-1))^2   with b=(t_new-1)/2
         = S*(cos + b)^2
  m      = [r1 < S*(1 - ctm_n)]   == [u > ctm_n]   (DVE tensor_tensor)
  g      = (h - S*b^2) * m        = m * S * (cos^2 + (t_new-1)*cos)
  out    = (r2 - S) + g           = S * where(m, cos*(t_new+cos), cos)
"""
import math

import numpy as np

import concourse.bass as bass
import concourse.bacc as bacc
import concourse.mybir as mybir
import concourse.tile as tile
from concourse.bass_utils import run_bass_kernel_spmd

fp32 = mybir.dt.float32
fp32r = mybir.dt.float32r
ALU = mybir.AluOpType
ACTF = mybir.ActivationFunctionType

MARGIN = 0.5
S = 64.0
SQS = math.sqrt(S)
COS_M = math.cos(MARGIN)
SIN_M = math.sin(MARGIN)
THRESHOLD = math.cos(math.pi - MARGIN)
MM = math.sin(math.pi - MARGIN) * MARGIN

N = 512          # batch rows
D = 512          # in_features
C = 100000       # classes
NCORES = 8
CS = 12800       # per-core (padded) column shard
NBLK = CS // 512  # 25 blocks of 512 columns

_prog_cache = {}


def _build_program():
    nc = bacc.Bacc(None, target_bir_lowering=False)

    embT_d = nc.dram_tensor("embT", [D, N], fp32, kind="ExternalInput")
    ksh_d = nc.dram_tensor("ksh", [D, CS], fp32, kind="ExternalInput")
    gk_d = nc.dram_tensor("gk", [D, N], fp32, kind="ExternalInput")
    t_d = nc.dram_tensor("tin", [1, 1], fp32, kind="ExternalInput")
    outT_d = nc.dram_tensor("outT", [CS, N], fp32, kind="ExternalOutput")
    ftl_d = nc.dram_tensor("ftl4", [128, 4], fp32, kind="ExternalOutput")

    ident_i = nc.inline_tensor(np.eye(128, dtype=np.float32), "ident_i")
    onescol_i = nc.inline_tensor(np.ones((128, 1), dtype=np.float32), "onescol_i")

    with tile.TileContext(nc) as tc:
        with (
            tc.tile_pool(name="const", bufs=1) as cp,
            tc.tile_pool(name="kin", bufs=2) as kin,
            tc.tile_pool(name="work", bufs=2) as wk,
            tc.tile_pool(name="small", bufs=2) as sm,
            tc.tile_pool(name="psraw", bufs=6, space="PSUM") as psraw,
            tc.tile_pool(name="psaux", bufs=1, space="PSUM") as psaux,
        ):
            # ---- constants / persistent tiles ----
            emb_sb = cp.tile([128, 4, N], fp32r, tag="emb")
            gk_sb = cp.tile([128, 4, N], fp32r, tag="gk")
            id_t = cp.tile([128, 128], fp32, tag="id")
            onesr = cp.tile([128, 1], fp32r, tag="onesr")
            ones_row = cp.tile([1, 128], fp32, tag="ones_row")
            biasS = cp.tile([128, 1], fp32, tag="biasS")
            bias2S = cp.tile([128, 1], fp32, tag="bias2S")
            t_sb = cp.tile([1, 1], fp32, tag="t_sb")
            thr_b4 = cp.tile([128, 4, N], fp32, tag="thr_b4")
            bh128 = cp.tile([128, 1], fp32, tag="bh128")
            qs128 = cp.tile([128, 1], fp32, tag="qs128")

            nc.gpsimd.dma_start(emb_sb[:], embT_d[:].rearrange("(g p) n -> p g n", p=128))
            nc.gpsimd.dma_start(gk_sb[:], gk_d[:].rearrange("(g p) n -> p g n", p=128))
            nc.sync.dma_start(id_t[:], ident_i[:])
            nc.gpsimd.dma_start(onesr[:], onescol_i[:])
            nc.sync.dma_start(t_sb[:], t_d[:])
            nc.vector.memset(ones_row[:], 1.0)
            nc.vector.memset(biasS[:], S)
            nc.vector.memset(bias2S[:], 2 * S)

            # ---- target path: tlraw4/n2g4 via gram diagonals (fp32r, ap=128) ----
            tlraw4 = sm.tile([128, 4], fp32, tag="tlraw4", bufs=1)
            n2g4 = sm.tile([128, 4], fp32, tag="n2g4", bufs=1)
            junk = sm.tile([128, 128], fp32, tag="junk", bufs=1)
            for j in range(4):
                G = psraw.tile([128, 128], fp32, tag="raw")
                for g in range(4):
                    nc.tensor.matmul(
                        G[:],
                        emb_sb[:, g, bass.ts(j, 128)],
                        gk_sb[:, g, bass.ts(j, 128)],
                        start=(g == 0),
                        stop=(g == 3),
                    )
                nc.vector.scalar_tensor_tensor(
                    junk[:], G[:], 1.0, id_t[:], ALU.bypass, ALU.mult,
                    accum_out=tlraw4[:, j : j + 1],
                )
            for j in range(4):
                H = psraw.tile([128, 128], fp32, tag="raw")
                for g in range(4):
                    nc.tensor.matmul(
                        H[:],
                        gk_sb[:, g, bass.ts(j, 128)],
                        gk_sb[:, g, bass.ts(j, 128)],
                        start=(g == 0),
                        stop=(g == 3),
                    )
                nc.vector.scalar_tensor_tensor(
                    junk[:], H[:], 1.0, id_t[:], ALU.bypass, ALU.mult,
                    accum_out=n2g4[:, j : j + 1],
                )

            # tl = clip(tlraw / sqrt(n2g), -1, 1)   [128, 4]
            rg = sm.tile([128, 4], fp32, tag="rg", bufs=1)
            nc.vector.reciprocal(rg[:], n2g4[:])
            invg = sm.tile([128, 4], fp32, tag="invg", bufs=1)
            nc.scalar.activation(invg[:], rg[:], ACTF.Sqrt)
            u_t = sm.tile([128, 4], fp32, tag="u_t", bufs=1)
            nc.vector.tensor_mul(u_t[:], tlraw4[:], invg[:])
            tl = sm.tile([128, 4], fp32, tag="tl", bufs=1)
            nc.vector.tensor_scalar(tl[:], u_t[:], -1.0, 1.0, ALU.max, ALU.min)

            # sin_sS = S*SIN_M*sqrt(1 - tl^2); ctm_s = S*COS_M*tl - sin_sS
            tl2 = sm.tile([128, 4], fp32, tag="tl2", bufs=1)
            nc.scalar.activation(tl2[:], tl[:], ACTF.Square)
            biasSin = cp.tile([128, 1], fp32, tag="biasSin")
            nc.vector.memset(biasSin[:], (S * SIN_M) ** 2)
            sin_sS = sm.tile([128, 4], fp32, tag="sin_sS", bufs=1)
            nc.scalar.activation(
                sin_sS[:], tl2[:], ACTF.Sqrt, bias=biasSin[:], scale=-((S * SIN_M) ** 2)
            )
            ctm_s = sm.tile([128, 4], fp32, tag="ctm_s", bufs=1)
            nc.vector.scalar_tensor_tensor(
                ctm_s[:], tl[:], S * COS_M, sin_sS[:], ALU.mult, ALU.subtract
            )

            # ftl_s = S * where(tl > THRESHOLD, ctm, tl - MM)   [128, 4]
            fm = sm.tile([128, 4], fp32, tag="fm", bufs=1)
            nc.vector.tensor_single_scalar(fm[:], tl[:], THRESHOLD, ALU.is_gt)
            soft_s = sm.tile([128, 4], fp32, tag="soft_s", bufs=1)
            nc.vector.tensor_scalar(soft_s[:], tl[:], MM, S, ALU.subtract, ALU.mult)
            dsel = sm.tile([128, 4], fp32, tag="dsel", bufs=1)
            nc.vector.tensor_sub(dsel[:], ctm_s[:], soft_s[:])
            esel = sm.tile([128, 4], fp32, tag="esel", bufs=1)
            nc.vector.tensor_mul(esel[:], dsel[:], fm[:])
            ftl_s = sm.tile([128, 4], fp32, tag="ftl_s", bufs=1)
            nc.vector.tensor_add(ftl_s[:], soft_s[:], esel[:])
            nc.sync.dma_start(ftl_d[:], ftl_s[:])

            # t_new = mean(tl)*0.01 + 0.99*t    [1, 1]
            mean_ps = psaux.tile([1, 4], fp32, tag="aux")
            nc.tensor.matmul(mean_ps[:], onesr[:].bitcast(fp32), tl[:], start=True, stop=True)
            j14 = sm.tile([1, 4], fp32, tag="j14", bufs=1)
            tsum = sm.tile([1, 1], fp32, tag="tsum", bufs=1)
            nc.vector.tensor_scalar(
                j14[:], mean_ps[:], 1.0, 0.0, ALU.mult, ALU.add, accum_out=tsum[:]
            )
            t99 = sm.tile([1, 1], fp32, tag="t99", bufs=1)
            nc.scalar.activation(t99[:], t_sb[:], ACTF.Copy, scale=0.99)
            t_new = sm.tile([1, 1], fp32, tag="t_new", bufs=1)
            nc.vector.scalar_tensor_tensor(
                t_new[:], tsum[:], 0.01 / N, t99[:], ALU.mult, ALU.add
            )

            # b = (t_new - 1)/2 ; bias_h = sqrt(S)*(b - 1) ; qS = S*b^2
            b11 = sm.tile([1, 1], fp32, tag="b11", bufs=1)
            nc.vector.tensor_scalar(b11[:], t_new[:], 1.0, 0.5, ALU.subtract, ALU.mult)
            bh11 = sm.tile([1, 1], fp32, tag="bh11", bufs=1)
            nc.vector.tensor_scalar(bh11[:], b11[:], 1.0, SQS, ALU.subtract, ALU.mult)
            qs11 = sm.tile([1, 1], fp32, tag="qs11", bufs=1)
            nc.scalar.activation(qs11[:], b11[:], ACTF.Square, scale=SQS)

            bh_ps = psaux.tile([128, 1], fp32, tag="aux")
            nc.tensor.matmul(bh_ps[:], ones_row[:], bh11[:], start=True, stop=True)
            nc.scalar.activation(bh128[:], bh_ps[:], ACTF.Copy)
            qs_ps = psaux.tile([128, 1], fp32, tag="aux")
            nc.tensor.matmul(qs_ps[:], ones_row[:], qs11[:], start=True, stop=True)
            nc.scalar.activation(qs128[:], qs_ps[:], ACTF.Copy)

            # thr row = S - ctm_s (per n), materialized as [128, 4, N] broadcast
            ctmrow_ps = psaux.tile([1, N], fp32, tag="aux2")
            for j in range(4):
                nc.tensor.matmul(
                    ctmrow_ps[0:1, bass.ts(j, 128)],
                    ctm_s[:, j : j + 1],
                    id_t[:],
                    start=True,
                    stop=True,
                )
            thrv = sm.tile([1, N], fp32, tag="thrv", bufs=1)
            nc.scalar.activation(thrv[:], ctmrow_ps[:], ACTF.Copy, bias=S, scale=-1.0)
            for j in range(4):
                tp = psraw.tile([128, N], fp32, tag="raw")
                nc.tensor.matmul(tp[:], ones_row[:], thrv[:], start=True, stop=True)
                nc.scalar.activation(thr_b4[:, j, :], tp[:], ACTF.Copy)

            # ---- main loop over 25 column blocks ----
            for blk in range(NBLK):
                kblk = kin.tile([128, 4, 512], fp32r, tag="kblk")
                nc.gpsimd.dma_start(
                    kblk[:],
                    ksh_d[:, bass.ts(blk, 512)].rearrange("(g p) c -> p g c", p=128),
                )
                # norms
                ksq = kin.tile([128, 4, 512], fp32r, tag="ksq")
                nc.scalar.activation(ksq[:], kblk[:], ACTF.Square)
                n2p = psaux.tile([1, 512], fp32, tag="aux2")
                for g in range(4):
                    nc.tensor.matmul(
                        n2p[:], onesr[:], ksq[:, g, :], start=(g == 0), stop=(g == 3)
                    )
                invsq = sm.tile([1, 512], fp32, tag="invsq")
                nc.vector.reciprocal(invsq[:], n2p[:])
                inv4P = psaux.tile([128, 4], fp32, tag="aux")
                for j in range(4):
                    nc.tensor.matmul(
                        inv4P[:, j : j + 1],
                        invsq[0:1, bass.ts(j, 128)],
                        ones_row[0:1, 0:1],
                        start=True,
                        stop=True,
                    )
                invS = sm.tile([128, 4], fp32, tag="invS")
                nc.scalar.activation(invS[:], inv4P[:], ACTF.Sqrt, scale=S * S)
                ninvS = sm.tile([128, 4], fp32, tag="ninvS")
                nc.vector.tensor_scalar_mul(ninvS[:], invS[:], -1.0)

                # main matmuls + ACT1 per c-tile
                r1 = wk.tile([128, 4, N], fp32, tag="r1")
                for j in range(4):
                    raw = psraw.tile([128, N], fp32, tag="raw")
                    for g in range(4):
                        nc.tensor.matmul(
                            raw[:],
                            kblk[:, g, bass.ts(j, 128)],
                            emb_sb[:, g, :],
                            start=(g == 0),
                            stop=(g == 3),
                        )
                    nc.scalar.activation(
                        r1[:, j, :], raw[:], ACTF.Relu,
                        bias=biasS[:], scale=ninvS[:, j : j + 1],
                    )
                r2 = wk.tile([128, 4, N], fp32, tag="r2")
                nc.scalar.activation(r2[:], r1[:], ACTF.Relu, bias=bias2S[:], scale=-1.0)
                h = wk.tile([128, 4, N], fp32, tag="h")
                nc.scalar.activation(h[:], r2[:], ACTF.Square, bias=bh128[:], scale=1.0 / SQS)
                m = wk.tile([128, 4, N], fp32, tag="m")
                nc.vector.tensor_tensor(m[:], r1[:], thr_b4[:], ALU.is_lt)
                g_t = wk.tile([128, 4, N], fp32, tag="g_t")
                nc.vector.scalar_tensor_tensor(
                    g_t[:], h[:], qs128[:], m[:], ALU.subtract, ALU.mult
                )
                o_t = wk.tile([128, 4, N], fp32, tag="o_t")
                nc.vector.scalar_tensor_tensor(
                    o_t[:], r2[:], S, g_t[:], ALU.subtract, ALU.add
                )
                nc.sync.dma_start(
                    outT_d[bass.ts(blk, 512), :].rearrange("(j p) n -> p j n", p=128),
                    o_t[:],
                )

    nc.finalize()
    return nc


def _get_program():
    if "nc" not in _prog_cache:
        _prog_cache["nc"] = _build_program()
    return _prog_cache["nc"]


def kernel(embeddings, label, kernel, t):
    embeddings = np.asarray(embeddings, dtype=np.float32)
    label = np.asarray(label)
    kern = np.asarray(kernel, dtype=np.float32)
    t = np.asarray(t, dtype=np.float32)

    embT = np.ascontiguousarray(embeddings.T)                      # [D, N]
    gk = np.ascontiguousarray(kern[:, label.astype(np.int64)])     # [D, N]
    t11 = t.reshape(1, 1)

    in_maps = []
    for i in range(NCORES):
        lo, hi = i * CS, (i + 1) * CS
        if hi <= C:
            ksh = np.ascontiguousarray(kern[:, lo:hi])
        else:
            pad = np.ones((D, hi - C), dtype=np.float32)
            ksh = np.ascontiguousarray(np.concatenate([kern[:, lo:C], pad], axis=1))
        in_maps.append({"embT": embT, "ksh": ksh, "gk": gk, "tin": t11})

    nc = _get_program()
    res = run_bass_kernel_spmd(nc, in_maps, core_ids=list(range(NCORES)))
    _prog_cache["last_res"] = res

    outT = np.concatenate([r["outT"] for r in res.results], axis=0)[:C]  # [C, N]
    out = np.ascontiguousarray(outT.T)                                   # [N, C]

    # host-side scatter of the target-column margin logits (row n at [n%128, n//128])
    ftl4 = res.results[0]["ftl4"]
    ftl = np.ascontiguousarray(ftl4.T).reshape(-1)                       # [N]
    out[np.arange(N), label.astype(np.int64)] = ftl
    return out

